# revision 54
# baseline (speedup 1.0000x reference)
# Trainium2 Bass kernel for ByteCombineCNN (conv byte-encoder + highway + projection).
#
# Structure (per core, data-parallel over batch*time, 8 groups of 512 samples):
#   - HBM I/O is bf16 both ways (host casts f32->bf16 on input and back on output),
#     halving DMA traffic vs f32.
#   - The input load is ONE DMA-xbar transpose per group, HBM->SBUF directly:
#     [512 samples, 512 feat] -> [128 part, 4 chunk, 512 samples] with the feature
#     permutation f = 4*p + j absorbed into the conv weight row order.
#   - The 7 conv branches (widths 1..7) x all valid positions are one matmul
#     X[s,512] @ Wbig[512,448] (K-chunk accumulated matmuls per 128-sample tile,
#     streaming only the columns whose byte-span intersects each K-chunk);
#     max over positions is a segmented free-dim reduce_max on DVE.
#   - h-transpose back to [channels, samples] runs on the PE (4 small transposes);
#     conv bias+relu rides the ACT per-partition bias operand.
#   - Highway layers: matmuls with K=112; relu/sigmoid on ACT, elementwise
#     sub/mul/add on Pool (SBUF-only: GPSIMD cannot touch PSUM).  The projection
#     runs with K=112 (bias added on host during the f32 upcast) and uses hT as
#     the stationary operand so outputs land in natural [samples, 512] layout.
#     PSUM->SBUF output copies are spread across DVE/ACT.
#   - The whole schedule is software-pipelined with a 3-group skew: iteration i
#     runs conv(i), highway-l0(i-1), highway-l1(i-2), proj(i-3) so no engine
#     stream blocks on the serial highway latency chain.
import numpy as np
import ml_dtypes

bf16 = ml_dtypes.bfloat16

B, T, BYTE_LEN, EMB = 8, 4096, 8, 64
FILTERS = [(1, 4), (2, 8), (3, 12), (4, 16), (5, 20), (6, 24), (7, 28)]
NPOS = [BYTE_LEN - w + 1 for w, _ in FILTERS]
LAST_DIM = 112
OUT_DIM = 512
FEAT = BYTE_LEN * EMB          # 512
CONV_COLS = sum(c * p for (w, c), p in zip(FILTERS, NPOS))  # 448
N_CORES = 8
S_PER_CORE = B * T // N_CORES  # 4096
GROUP = 512                    # samples per group
NG = S_PER_CORE // GROUP       # 8
NST = GROUP // 128             # 4 subtiles per group
HMI_COLS = sum(c * ((BYTE_LEN - w + 1 + 1) // 2) for w, c in FILTERS)  # 248

_cache = {}

# engine assignment knobs (tuned against the CoreSim cost model)
DEFAULT_CFG = dict(
    # engine per (pr, filter) reduce ('v' = DVE; Pool can't do free-axis reduce)
    reduce_eng="vvvvvvv" + "vvvvvvv",
    # engine per proj psum->sbuf copy (4 subtiles): 'v'/'a' (PSUM readers), or
    # 's' = split half ACT half DVE.  Pool cannot access PSUM (HW restriction).
    copy_eng="asas",
    # engine for highway sub / mul / add per layer (SBUF-only ops: Pool legal)
    hw_sub="pp", hw_mul="pp", hw_add="pp",
    # psum layout: "scr" = ht_ps/o_ps share a 2-buf pool (conv4+scr2+pg2);
    # "pgo" = o_ps shares the pg pool, ht_ps gets its own pool
    psum="pgo", pg_bufs=3, ht_bufs=1,
    xt_bufs=3, hraw_bufs=2, out_bufs=2, act_bufs=2, htp_bufs=3,
    # conv matmul emission: "dense" streams all 448 cols for each K-chunk;
    # "skip" streams only the columns whose byte-span intersects the chunk.
    # skip looks good in the cost model but is slower on HW: it multiplies
    # LDWEIGHTS count and the position-major layout it needs makes the
    # segmented reduce_max read strided.  Layout follows this flag.
    conv="dense",
    # pipeline truncation for HW timing bisection (6 = full kernel):
    # 1=loads 2=+conv/reduce 3=+transpose/relu 4=+highway 5=+proj 6=+store
    stage=6,
    # drop the segmented reduces (timing bisection only; breaks numerics)
    noreduce=False,
    # batch the xbar loads over two groups (halves load DMA instruction count)
    load2=False,
    # split the LAST group's highway layers into N column slices so the
    # pipeline-drain dependency chains run on short slices instead of the
    # full 512-sample group (1 = off, identical stream to the tuned default)
    tail_slices=1,
    # "direct": one segmented reduce_max per (pr, filter) straight from PSUM.
    # "ttmax": pairwise tensor_tensor max (two half-position operands, middle
    # position read twice for odd counts - max is idempotent) into SBUF bf16,
    # then a short reduce from SBUF; halves the elements each DVE pass streams.
    redmode="direct",
)


def _conv_pieces():
    """Per K-chunk, the merged contiguous column ranges it contributes to.

    Columns are position-major within each filter block (col = off + p*c + cc),
    so the positions whose byte-span [p, p+w) intersects chunk kc form one
    contiguous range per filter; ranges of adjacent filters merge when they
    touch.  PSUM start/stop is bank-granular (start zero-fills the bank with
    per-byte zero-on-first-touch), so pieces carry no flags: only the first
    instruction into the bank sets start, the last sets stop.
    Returns {kc: [(col0, col1), ...]}.
    """
    out = {}
    for kc in range(4):
        ranges = []  # [c0, c1, is_first_touch]
        off = 0
        for (w, c), p_i in zip(FILTERS, NPOS):
            lo = max(0, 2 * kc - w + 1)
            hi = min(p_i - 1, 2 * kc + 1)
            # split at p = 2kc: below it this chunk accumulates onto earlier
            # chunks, from it on this chunk is the column's first touch
            for (a, b) in ((lo, min(hi, 2 * kc - 1)), (max(lo, 2 * kc), hi)):
                if a > b:
                    continue
                first = a >= 2 * kc
                c0, c1 = off + a * c, off + (b + 1) * c
                if ranges and ranges[-1][1] == c0 and ranges[-1][2] == first:
                    ranges[-1][1] = c1
                else:
                    ranges.append([c0, c1, first])
            off += c * p_i
        out[kc] = [(r[0], r[1]) for r in ranges]
    return out


def _build(reps=1, cfg=None):
    import concourse.mybir as mybir
    import concourse.tile as tile
    from concourse import bacc
    from contextlib import ExitStack

    cfg = dict(DEFAULT_CFG, **(cfg or {}))
    dt = mybir.dt
    nc = bacc.Bacc("TRN2", target_bir_lowering=False, debug=False)

    featd = nc.dram_tensor("features", [NG, GROUP, FEAT], dt.bfloat16,
                           kind="ExternalInput").ap()
    wbig_d = nc.dram_tensor("wbig", [128, 4 * CONV_COLS], dt.bfloat16,
                            kind="ExternalInput").ap()
    hwT_d = nc.dram_tensor("hwT", [112, 448], dt.bfloat16, kind="ExternalInput").ap()
    pwT_d = nc.dram_tensor("pwT", [112, 512], dt.bfloat16, kind="ExternalInput").ap()
    cbias_d = nc.dram_tensor("cbias", [112, 1], dt.float32, kind="ExternalInput").ap()
    hbias_d = nc.dram_tensor("hbias", [112, 4], dt.float32, kind="ExternalInput").ap()
    ident_d = nc.dram_tensor("ident", [128, 128], dt.bfloat16, kind="ExternalInput").ap()
    outp = nc.dram_tensor("out", [S_PER_CORE, OUT_DIM], dt.bfloat16,
                          kind="ExternalOutput").ap()

    outv = outp.rearrange("(g st p) o -> g p st o", st=NST, p=128)

    def eng(ch):
        return {"v": nc.vector, "p": nc.gpsimd, "a": nc.scalar}[ch]

    with tile.TileContext(nc) as tc, ExitStack() as ctx:
        const = ctx.enter_context(tc.tile_pool(name="const", bufs=1))
        wbig_sb = const.tile([128, 4, CONV_COLS], dt.bfloat16, name="wbig_sb")
        hwT_sb = const.tile([112, 448], dt.bfloat16, name="hwT_sb")
        pwT_sb = const.tile([112, 512], dt.bfloat16, name="pwT_sb")
        cbias_sb = const.tile([112, 1], dt.float32, name="cbias_sb")
        hbias_sb = const.tile([112, 4], dt.float32, name="hbias_sb")
        ident_sb = const.tile([128, 128], dt.bfloat16, name="ident_sb")

        def load_consts(part=None):
            # wbig first (conv needs it); remaining consts slot in after the
            # second group load so the pipeline fill isn't DMA-starved
            if part in (None, 1):
                nc.sync.dma_start(out=wbig_sb[:],
                                  in_=wbig_d.rearrange("p (k c) -> p k c", k=4))
            if part in (None, 2):
                nc.sync.dma_start(out=ident_sb[:], in_=ident_d)
                nc.sync.dma_start(out=cbias_sb[:], in_=cbias_d)
                nc.sync.dma_start(out=hwT_sb[:], in_=hwT_d)
                nc.sync.dma_start(out=hbias_sb[:], in_=hbias_d)
                nc.sync.dma_start(out=pwT_sb[:], in_=pwT_d)

        xt_pool = ctx.enter_context(tc.tile_pool(name="xt", bufs=cfg["xt_bufs"]))
        if cfg["redmode"] == "ttmax":
            hmi_pool = ctx.enter_context(tc.tile_pool(name="hmi", bufs=2))
        conv_ps_pool = ctx.enter_context(tc.tile_pool(name="conv_ps", bufs=2, space="PSUM"))
        hraw_pool = ctx.enter_context(tc.tile_pool(name="hraw", bufs=cfg["hraw_bufs"]))
        ht_pool = ctx.enter_context(tc.tile_pool(name="ht", bufs=cfg["htp_bufs"]))
        act_pool = ctx.enter_context(tc.tile_pool(name="act", bufs=cfg["act_bufs"]))
        out_pool = ctx.enter_context(tc.tile_pool(name="outsb", bufs=cfg["out_bufs"]))
        if cfg["psum"] == "scr":
            scr_ps_pool = ctx.enter_context(
                tc.tile_pool(name="scr_ps", bufs=2, space="PSUM"))
            pg_ps_pool = ctx.enter_context(
                tc.tile_pool(name="pg_ps", bufs=1, space="PSUM"))
            ht_ps_pool, o_ps_pool = scr_ps_pool, scr_ps_pool
            ht_tag = o_tag = "scr"
            pg_tag = None
        elif cfg["psum"] == "pgo":  # o_ps rides the pg rotation, ht_ps standalone
            pg_ps_pool = ctx.enter_context(
                tc.tile_pool(name="pg_ps", bufs=cfg["pg_bufs"], space="PSUM"))
            ht_ps_pool = ctx.enter_context(
                tc.tile_pool(name="ht_ps", bufs=cfg["ht_bufs"], space="PSUM"))
            o_ps_pool = pg_ps_pool
            ht_tag = None
            o_tag = pg_tag = "pg"
        else:  # "cht": ht_ps rides the conv rotation; pg/o pool gets 4 slots
            pg_ps_pool = ctx.enter_context(
                tc.tile_pool(name="pg_ps", bufs=cfg["pg_bufs"], space="PSUM"))
            ht_ps_pool = conv_ps_pool
            o_ps_pool = pg_ps_pool
            ht_tag = "cv"
            o_tag = pg_tag = "pg"

        def stage_load(g):
            # one xbar transpose HBM->SBUF, bf16; xt[p, j, s] = X[128j + p, s]
            # (wbig row order matches).
            xt = xt_pool.tile([128, 4, GROUP], dt.bfloat16, name="xt")
            nc.sync.dma_start_transpose(out=xt[:], in_=featd[g])
            return xt

        def stage_load2(g):
            # batched variant: one xbar transpose covering two groups
            xt2 = xt_pool.tile([128, 4, 2 * GROUP], dt.bfloat16, name="xt2")
            nc.sync.dma_start_transpose(
                out=xt2[:],
                in_=featd[g:g + 2].rearrange("t s f -> (t s) f"),
            )
            return xt2

        pieces = _conv_pieces()

        def stage_conv(xt_base):
            # conv as dense matmul + segmented maxpool (per subtile pair)
            xt, base = xt_base
            hraw = hraw_pool.tile([128, NST, LAST_DIM], dt.bfloat16, name="hraw")
            for pr in range(NST // 2):
                conv_ps = conv_ps_pool.tile([128, 2, 512], dt.float32, name="conv_ps",
                                            tag=ht_tag if cfg["psum"] == "cht" else None)
                for sub in range(2):
                    st = pr * 2 + sub
                    lhs = [xt[:, kc, base + st * 128:base + (st + 1) * 128]
                           for kc in range(4)]
                    if cfg["conv"] == "dense":
                        for kc in range(4):
                            nc.tensor.matmul(
                                conv_ps[:, sub, 0:CONV_COLS],
                                lhsT=lhs[kc],
                                rhs=wbig_sb[:, kc, :],
                                start=(kc == 0),
                                stop=(kc == 3),
                            )
                    else:
                        n_pieces = sum(len(v) for v in pieces.values())
                        k = 0
                        for kc in range(4):
                            for (c0, c1) in pieces[kc]:
                                nc.tensor.matmul(
                                    conv_ps[:, sub, c0:c1],
                                    lhsT=lhs[kc],
                                    rhs=wbig_sb[:, kc, c0:c1],
                                    start=(k == 0),
                                    stop=(k == n_pieces - 1),
                                )
                                k += 1
                off = 0
                offc = 0
                hoff = 0
                if cfg["noreduce"]:
                    continue
                if cfg["redmode"] == "ttmax":
                    hmi = hmi_pool.tile([128, 2, HMI_COLS], dt.bfloat16, name="hmi")
                for fi, ((w, c), p_i) in enumerate(zip(FILTERS, NPOS)):
                    if cfg["conv"] == "dense":
                        # channel-major (cc, p): contiguous positions per channel
                        seg = conv_ps[:, :, off:off + c * p_i].rearrange(
                            "a b (cc p) -> a b cc p", p=p_i
                        )
                    else:
                        seg = conv_ps[:, :, off:off + c * p_i].rearrange(
                            "a b (p cc) -> a b cc p", cc=c
                        )
                    red_eng = eng(cfg["reduce_eng"][pr * 7 + fi])
                    if cfg["redmode"] == "ttmax":
                        ce, fl = (p_i + 1) // 2, p_i // 2
                        half = hmi[:, :, hoff:hoff + c * ce].rearrange(
                            "a b (cc q) -> a b cc q", q=ce
                        )
                        red_eng.tensor_max(half, seg[:, :, :, 0:ce],
                                           seg[:, :, :, fl:fl + ce])
                        src = half
                    else:
                        src = seg
                    red_eng.tensor_reduce(
                        out=hraw[:, pr * 2:(pr + 1) * 2, offc:offc + c],
                        in_=src,
                        axis=mybir.AxisListType.X,
                        op=mybir.AluOpType.max,
                    )
                    off += c * p_i
                    offc += c
                    hoff += c * ((p_i + 1) // 2)
            return hraw

        def stage_ht_transpose(hraw):
            # transpose h to [c, s] on PE
            ht_ps = ht_ps_pool.tile([112, NST, 128], dt.bfloat16, name="ht_ps",
                                    tag=ht_tag)
            if cfg["psum"] == "cht":
                # keep the shared conv/ht rotation at 4 allocs per iteration so
                # every tensor name lands in a stable slot
                ht_ps_pool.tile([112, NST, 128], dt.bfloat16, name="cv_pad",
                                tag=ht_tag)
            for st in range(NST):
                nc.tensor.transpose(ht_ps[:, st, :], hraw[:, st, :], ident_sb[:])
            return ht_ps

        def stage_ht_relu(ht_ps):
            # conv bias+relu on ACT
            hT = ht_pool.tile([112, GROUP], dt.bfloat16, name="hT0")
            nc.scalar.activation(
                hT[:], ht_ps.rearrange("a b c -> a (b c)"),
                mybir.ActivationFunctionType.Relu, bias=cbias_sb[:],
            )
            return hT

        def stage_hw(l, hT, nsl=1):
            # one highway layer (K=112; biases via ACT per-partition bias);
            # nsl > 1 runs the chain on column slices to shorten its latency
            p_ps = pg_ps_pool.tile([112, GROUP], dt.float32, name=f"p_ps{l}",
                                   tag=pg_tag)
            g_ps = pg_ps_pool.tile([112, GROUP], dt.float32, name=f"g_ps{l}",
                                   tag=pg_tag)
            rp = act_pool.tile([112, GROUP], dt.bfloat16, name=f"rp{l}")
            gs = act_pool.tile([112, GROUP], dt.bfloat16, name=f"gs{l}")
            d = act_pool.tile([112, GROUP], dt.bfloat16, name=f"d{l}")
            e = act_pool.tile([112, GROUP], dt.bfloat16, name=f"e{l}")
            hT_next = ht_pool.tile([112, GROUP], dt.bfloat16,
                                   name="hT1" if l == 0 else "hT_fin")
            for s in range(nsl):
                sl = slice(s * GROUP // nsl, (s + 1) * GROUP // nsl)
                nc.tensor.matmul(p_ps[:, sl], lhsT=hwT_sb[:, l * 224:l * 224 + 112],
                                 rhs=hT[0:112, sl], start=True, stop=True)
                nc.tensor.matmul(g_ps[:, sl],
                                 lhsT=hwT_sb[:, l * 224 + 112:l * 224 + 224],
                                 rhs=hT[0:112, sl], start=True, stop=True)
                nc.scalar.activation(rp[:, sl], p_ps[:, sl],
                                     mybir.ActivationFunctionType.Relu,
                                     bias=hbias_sb[:, 2 * l:2 * l + 1])
                nc.scalar.activation(gs[:, sl], g_ps[:, sl],
                                     mybir.ActivationFunctionType.Sigmoid,
                                     bias=hbias_sb[:, 2 * l + 1:2 * l + 2])
                eng(cfg["hw_sub"][l]).tensor_sub(d[:, sl], hT[0:112, sl], rp[:, sl])
                eng(cfg["hw_mul"][l]).tensor_mul(e[:, sl], gs[:, sl], d[:, sl])
                eng(cfg["hw_add"][l]).tensor_add(hT_next[0:112, sl], e[:, sl],
                                                 rp[:, sl])
            return hT_next

        def stage_proj(g, hT):
            # out[s, 512] directly (hT stationary, K=112; proj bias added on host)
            osb = out_pool.tile([128, NST, OUT_DIM], dt.bfloat16, name="osb")
            for st in range(NST):
                o_ps = o_ps_pool.tile([128, OUT_DIM], dt.float32, name="o_ps",
                                      tag=o_tag)
                nc.tensor.matmul(o_ps[:], lhsT=hT[:, st * 128:(st + 1) * 128],
                                 rhs=pwT_sb[:], start=True, stop=True)
                ce = cfg["copy_eng"][st]
                if ce == "a":
                    nc.scalar.copy(out=osb[:, st, :], in_=o_ps[:])
                elif ce == "s":
                    nc.scalar.copy(out=osb[:, st, 0:256], in_=o_ps[:, 0:256])
                    nc.vector.tensor_copy(out=osb[:, st, 256:512],
                                          in_=o_ps[:, 256:512])
                else:
                    eng(ce).tensor_copy(out=osb[:, st, :], in_=o_ps[:])
            if cfg["stage"] >= 6:
                nc.sync.dma_start(out=outv[g], in_=osb[:])

        def pg_align(allocs):
            # pad the pg-tag rotation to a multiple of its buf count so every
            # tensor name lands in a stable PSUM slot each iteration
            if pg_tag is not None:
                for _ in range((-allocs) % cfg["pg_bufs"]):
                    pg_ps_pool.tile([112, GROUP], dt.float32, name="pg_pad",
                                    tag=pg_tag)

        def pipeline(first):
            # software-pipelined schedule: iteration i runs conv(i),
            # hw-l0(i-1), hw-l1(i-2), proj(i-3) so no engine stream ever
            # blocks on the serial highway latency chain.
            xts, ht_pss, hT0s, hT1s, fins = {}, {}, {}, {}, {}
            if cfg["load2"]:
                t = stage_load2(0)
                xts[0], xts[1] = (t, 0), (t, GROUP)
            else:
                xts[0] = (stage_load(0), 0)
            if first:
                load_consts(part=1)
            stage = cfg["stage"]
            for i in range(NG + 3):
                pg_allocs = 0
                if cfg["load2"]:
                    if i % 2 == 0 and i + 2 < NG:
                        t = stage_load2(i + 2)
                        xts[i + 2], xts[i + 3] = (t, 0), (t, GROUP)
                elif i + 1 < NG:
                    xts[i + 1] = (stage_load(i + 1), 0)
                if i == 0 and first:
                    load_consts(part=2)
                if i < NG and stage >= 2:
                    hraw = stage_conv(xts.pop(i))
                if 0 <= i - 1 < NG and stage >= 4:
                    nsl = cfg["tail_slices"] if i - 1 == NG - 1 else 1
                    hT1s[i - 1] = stage_hw(0, hT0s.pop(i - 1), nsl)
                    pg_allocs += 2
                if 0 <= i - 2 < NG and stage >= 4:
                    nsl = cfg["tail_slices"] if i - 2 == NG - 1 else 1
                    fins[i - 2] = stage_hw(1, hT1s.pop(i - 2), nsl)
                    pg_allocs += 2
                if 0 <= i - 3 < NG and stage >= 5:
                    stage_proj(i - 3, fins.pop(i - 3) if stage >= 4 else None)
                    pg_allocs += 4
                if i < NG and stage >= 3:
                    ht_pss[i] = stage_ht_transpose(hraw)
                    hT0s[i] = stage_ht_relu(ht_pss.pop(i))
                pg_align(pg_allocs)

        if reps == 1:
            pipeline(first=True)
        else:
            load_consts()
            with tc.For_i(0, reps, 1):
                pipeline(first=False)

    nc.compile()
    return nc


def _prep_weights(inputs):
    W = np.zeros((FEAT, CONV_COLS), np.float32)
    cb = np.zeros(LAST_DIM, np.float32)
    off = 0
    offc = 0
    for i, ((w, c), p_i) in enumerate(zip(FILTERS, NPOS)):
        cw = np.asarray(inputs[f"conv_w{i+1}"], np.float32)  # [c, EMB, w]
        for p in range(p_i):
            for k in range(w):
                byte = p + k
                if DEFAULT_CFG["conv"] == "dense":
                    # channel-major within the filter block: col = cc*p_i + p
                    W[byte * EMB:(byte + 1) * EMB,
                      off + p:off + c * p_i:p_i] = cw[:, :, k].T
                else:
                    # position-major: col = p*c + cc
                    W[byte * EMB:(byte + 1) * EMB,
                      off + p * c:off + (p + 1) * c] = cw[:, :, k].T
        cb[offc:offc + c] = np.asarray(inputs[f"conv_b{i+1}"], np.float32)
        off += c * p_i
        offc += c
    # xbar transpose folds the feature axis as f = p + 128*j: chunk j holds
    # feature rows [128j, 128j+128) on partition p = f % 128
    wbig = np.ascontiguousarray(
        W.reshape(4, 128, CONV_COLS).transpose(1, 0, 2).reshape(128, 4 * CONV_COLS)
    ).astype(bf16)
    hwT = np.concatenate([np.asarray(inputs["hw_w1"], np.float32).T,
                          np.asarray(inputs["hw_w2"], np.float32).T], 1)
    hwT = np.ascontiguousarray(hwT).astype(bf16)  # [112, 448]
    pwT = np.ascontiguousarray(np.asarray(inputs["proj_w"], np.float32).T).astype(bf16)
    hb1 = np.asarray(inputs["hw_b1"], np.float32)
    hb2 = np.asarray(inputs["hw_b2"], np.float32)
    hbias = np.stack([hb1[:112], hb1[112:], hb2[:112], hb2[112:]], 1)  # [112, 4]
    hbias = np.ascontiguousarray(hbias)
    return wbig, hwT, pwT, cb.reshape(112, 1), hbias


def _prep_inputs(inputs):
    wbig, hwT, pwT, cb, hbias = _prep_weights(inputs)
    ident = np.eye(128, dtype=bf16)
    feats = np.asarray(inputs["features"], np.float32).reshape(B * T, FEAT).astype(bf16)
    in_maps = []
    for c in range(N_CORES):
        in_maps.append({
            "features": np.ascontiguousarray(
                feats[c * S_PER_CORE:(c + 1) * S_PER_CORE].reshape(NG, GROUP, FEAT)),
            "wbig": wbig, "hwT": hwT, "pwT": pwT, "cbias": cb,
            "hbias": hbias, "ident": ident,
        })
    return in_maps


def kernel(**inputs) -> np.ndarray:
    from concourse.bass_utils import run_bass_kernel_spmd

    if "nc" not in _cache:
        _cache["nc"] = _build()
    nc = _cache["nc"]

    in_maps = _prep_inputs(inputs)
    res = run_bass_kernel_spmd(nc, in_maps, core_ids=list(range(N_CORES)))
    out = np.concatenate([res.results[c]["out"] for c in range(N_CORES)], 0)
    out = out.astype(np.float32) + np.asarray(inputs["proj_b"], np.float32)
    return out.reshape(B, T, OUT_DIM)
